# revision 18
# baseline (speedup 1.0000x reference)
"""Trainium2 Bass kernel for nn_DetailCapture (deformable-conv detail-capture block).

Sharding: 8 cores = batch (2) x row-blocks (4 x 32 rows). Each core computes its
[b, :, y0:y0+32, :] output slice from host-staged per-core input slabs (halos are
baked into the slabs, so no collectives are needed).

Per-core device pipeline (per 32-row block):
  1. Offset convs for all 3 branches via PE matmuls (9 taps x 2 k-tiles, PSUM acc)
  2. PE-transpose offsets to [pixel, 54] layout
  3. Floor/frac/bilinear-weight/index tables on DVE (is_ge floor chains),
     bilinear weights cast to bf16
  4. Per row x branch: indirect-DMA gather of the 4 bilinear taps from an fp8
     image table (cast to bf16 during DMA); 9 gathers/row-branch
  5. Bilinear apply: 36 per-partition-scalar products in-place on the gathered
     tile, split across DVE (tensor_scalar) and ACT (copy-scale) engines;
     corner-sum via 3 wide strided TTs; depthwise defw via 1 wide TT;
     tap-sum via PE identity matmuls into PSUM
  6. LayerNorm stats via ACT accum_out + tensor_tensor_reduce; normalize+gelu
     fused into single ACT Gelu ops (exact erf gelu)
  7. 1x1 conv (PE), branch sum, LN, per-pixel MLP (bf16 PE matmuls), residual, LN
"""
import sys
import numpy as np

sys.path.insert(0, "/opt/trn_rl_repo")

import concourse.bass as bass
import concourse.bacc as bacc
import concourse.mybir as mybir
import concourse.tile as tile
from concourse.bass import AP

P = 128
B, C, H, W = 2, 256, 128, 128
RB = 32                # rows per core block
NCORES = 8
DILS = (1, 9, 12)
NBR = 3
K = 9
HALO = 12              # conv halo (max dil)
SROWS = RB + 2 * HALO  # 56 rows in conv slab
WP = W + 2 * HALO      # 152 padded width
TPAD = 16              # xT slab vertical pad rows (each side)
TROWS = (RB + 2 * TPAD) * W
TR_ALLOC = TROWS + 2 * W
A_OP = mybir.AluOpType
F32 = mybir.dt.float32
BF16 = mybir.dt.bfloat16
FP8 = mybir.dt.float8e4
I32 = mybir.dt.int32
I16 = mybir.dt.int16
AF = mybir.ActivationFunctionType

EPS = 1e-6
USE_FP8 = False        # gather table dtype (fp8 halves gather HBM traffic)
NACT = 17              # of the 36 bilinear products, how many go to ACT engine

_COMPILED = None


def build_program():
    nc = bacc.Bacc(None, target_bir_lowering=False, debug=False,
                   num_swdge_queues=4)
    XDT = FP8 if USE_FP8 else BF16

    # ---------------- DRAM I/O (host-staged layouts) ----------------
    d_xslab = nc.dram_tensor("xslab", [2, P, SROWS * WP], BF16, kind="ExternalInput")
    d_xts = [nc.dram_tensor(f"xt{i}", [TR_ALLOC, 2 * C], XDT,
                            kind="ExternalInput") for i in range(NBR * K)]
    d_offw = nc.dram_tensor("offw", [P, NBR * K * 2 * 18], BF16, kind="ExternalInput")
    d_convw = nc.dram_tensor("convw", [P, 4 * P], BF16, kind="ExternalInput")
    d_convb = nc.dram_tensor("convb", [P, 2], F32, kind="ExternalInput")
    d_w1T = nc.dram_tensor("w1T", [P, 2 * 512], BF16, kind="ExternalInput")
    d_b1row = nc.dram_tensor("b1row", [1, 512], BF16, kind="ExternalInput")
    d_w2T = nc.dram_tensor("w2T", [P, 4 * C], BF16, kind="ExternalInput")
    d_b2row = nc.dram_tensor("b2row", [1, C], BF16, kind="ExternalInput")
    d_ybrel = nc.dram_tensor("ybrel", [P, NBR * RB * K], BF16, kind="ExternalInput")
    d_ybabs = nc.dram_tensor("ybabs", [P, NBR * RB * K], BF16, kind="ExternalInput")
    d_xvb = nc.dram_tensor("xvb", [P, NBR * RB * K], BF16, kind="ExternalInput")
    d_ident = nc.dram_tensor("identf", [P, P], F32, kind="ExternalInput")
    d_identb = nc.dram_tensor("identb", [P, P], BF16, kind="ExternalInput")
    d_ones1 = nc.dram_tensor("ones1", [1, P], BF16, kind="ExternalInput")
    d_out = nc.dram_tensor("out", [RB * P, C], F32, kind="ExternalOutput")

    with tile.TileContext(nc) as tc:
        import contextlib
        ctx = contextlib.ExitStack()
        with ctx:
            cpool = ctx.enter_context(tc.tile_pool(name="const", bufs=1))
            spool = ctx.enter_context(tc.tile_pool(name="slab", bufs=1))
            tpool = ctx.enter_context(tc.tile_pool(name="tables", bufs=1))
            wpool = ctx.enter_context(tc.tile_pool(name="work", bufs=1))
            gpool = ctx.enter_context(tc.tile_pool(name="gath", bufs=4))
            apool = ctx.enter_context(tc.tile_pool(name="apply", bufs=2))
            mpool = ctx.enter_context(tc.tile_pool(name="mlp", bufs=2))
            pspool = ctx.enter_context(tc.tile_pool(name="ps", bufs=2, space="PSUM"))

            # ---- constants ----
            def load_const(name, dram, shape, dtype):
                t = cpool.tile(shape, dtype, tag=name, name=name)
                nc.sync.dma_start(t[:], dram[:])
                return t

            ident = load_const("ident", d_ident, [P, P], F32)
            identb = load_const("identb", d_identb, [P, P], BF16)
            ones1 = load_const("ones1", d_ones1, [1, P], BF16)
            convb = load_const("convb", d_convb, [P, 2], F32)
            offw = load_const("offw", d_offw, [P, NBR * K * 2 * 18], BF16)
            convw = load_const("convw", d_convw, [P, 4 * P], BF16)
            w1T = load_const("w1T", d_w1T, [P, 2 * 512], BF16)
            w2T = load_const("w2T", d_w2T, [P, 4 * C], BF16)
            b1row = load_const("b1row", d_b1row, [1, 512], BF16)
            b2row = load_const("b2row", d_b2row, [1, C], BF16)
            ybrel = load_const("ybrel", d_ybrel, [P, NBR * RB * K], BF16)
            ybabs = load_const("ybabs", d_ybabs, [P, NBR * RB * K], BF16)
            xvb = load_const("xvb", d_xvb, [P, NBR * RB * K], BF16)

            xslab = [spool.tile([P, SROWS, WP], BF16, tag=f"xs{kt}", name=f"xs{kt}") for kt in range(2)]
            for kt in range(2):
                nc.sync.dma_start(
                    xslab[kt][:], d_xslab[kt].rearrange("p (r w) -> p r w", w=WP))

            def offw_ap(br, tap, kt):
                base = ((br * K + tap) * 2 + kt) * 18
                return offw[:, base:base + 18]

            # ---- stage 1+2: offset convs -> transpose -> offT [128, RB, 96] ----
            offT = tpool.tile([P, RB, 96], F32)
            nc.vector.memset(offT[:], 0.0)
            for chunk in range(RB // 4):          # 4 rows = 512 px per chunk
                ps_off = pspool.tile([96, 512], F32, tag="ps2k", bufs=2)
                for br in range(NBR):
                    dil = DILS[br]
                    for tap in range(K):
                        dy, dx = tap // 3, tap % 3
                        srow = 4 * chunk + HALO + (dy - 1) * dil
                        scol = HALO + (dx - 1) * dil
                        for kt in range(2):
                            nc.tensor.matmul(
                                ps_off[br * 32:br * 32 + 18, :],
                                lhsT=offw_ap(br, tap, kt),
                                rhs=xslab[kt][:, srow:srow + 4, scol:scol + P],
                                start=(tap == 0 and kt == 0),
                                stop=(tap == K - 1 and kt == 1),
                            )
                osb = wpool.tile([96, 512], F32, tag="osb")
                for br in range(NBR):
                    nc.any.tensor_copy(osb[br * 32:br * 32 + 18, :],
                                       ps_off[br * 32:br * 32 + 18, :])
                for sub in range(4):
                    r = chunk * 4 + sub
                    for br in range(NBR):
                        ps_t = pspool.tile([P, 18], F32, tag="ps_sm", bufs=1)
                        nc.tensor.transpose(
                            ps_t[:], osb[br * 32:br * 32 + 18, sub * P:(sub + 1) * P],
                            ident[br * 32:br * 32 + 18, br * 32:br * 32 + 18])
                        nc.any.tensor_copy(offT[:, r, br * 32:br * 32 + 18], ps_t[:])

            # ---- stage 3: per-branch tables ----
            # 4 bilinear corner weights (f32: tensor_scalar/ACT scale operands)
            sb = [tpool.tile([P, NBR, RB, K], F32, tag=f"sb{i}", name=f"sb{i}")
                  for i in range(4)]          # order: s00, s10, s01, s11
            idx0 = tpool.tile([P, NBR, RB, K], I32)

            NE = RB * K
            _shp = [[P, RB, K]]

            def wt(tag):
                return wpool.tile(list(_shp[0]), F32, tag=tag, name=tag)

            def floor_chain(dst, src_ap):
                nc.vector.tensor_scalar(
                    out=dst[:], in0=src_ap, scalar1=-2.0, scalar2=-3.0,
                    op0=A_OP.is_ge, op1=A_OP.add)
                for t in (-1.0, 0.0, 1.0, 2.0):
                    cmp_t = wt("cmp")
                    nc.vector.tensor_scalar(
                        out=cmp_t[:], in0=src_ap, scalar1=t, scalar2=None,
                        op0=A_OP.is_ge)
                    nc.vector.tensor_tensor(
                        out=dst[:], in0=dst[:], in1=cmp_t[:], op=A_OP.add)

            def bound_mask(dst, src, lo, hi):
                m2 = wt("mtmp")
                nc.vector.tensor_scalar(out=dst[:], in0=src[:], scalar1=lo,
                                        scalar2=None, op0=A_OP.is_ge)
                nc.vector.tensor_scalar(out=m2[:], in0=src[:], scalar1=hi,
                                        scalar2=None, op0=A_OP.is_le)
                nc.vector.tensor_tensor(out=dst[:], in0=dst[:], in1=m2[:],
                                        op=A_OP.mult)

            part_dim = offT[:].ap[0]
            base_off = offT[:].offset
            # two-pass table build: rows 0:8 first (unblocks the row pipeline
            # after stage-1 chunks 0-1), then rows 8:32
            for (roff, nrows) in ((0, 8), (8, RB - 8)):
              _shp[0] = [P, nrows, K]
              NEr = nrows * K
              for br in range(NBR):
                offy_v = AP(offT.tensor, base_off + roff * 96 + br * 32,
                            [part_dim, [96, nrows], [2, K]])
                offx_v = AP(offT.tensor, base_off + roff * 96 + br * 32 + 1,
                            [part_dim, [96, nrows], [2, K]])
                cst = lambda t: t[:, br * NE + roff * K:
                                  br * NE + roff * K + NEr].rearrange(
                    "p (r k) -> p r k", k=K)
                ybrel_v = cst(ybrel)
                ybabs_v = cst(ybabs)
                xvb_v = cst(xvb)

                fy = wt("fy")
                floor_chain(fy, offy_v)
                fx = wt("fx")
                floor_chain(fx, offx_v)

                ay = wt("ay")
                nc.vector.tensor_tensor(out=ay[:], in0=offy_v, in1=fy[:],
                                        op=A_OP.subtract)
                ax = wt("ax")
                nc.vector.tensor_tensor(out=ax[:], in0=offx_v, in1=fx[:],
                                        op=A_OP.subtract)
                y0a = wt("y0a")
                nc.vector.tensor_tensor(out=y0a[:], in0=ybabs_v, in1=fy[:],
                                        op=A_OP.add)
                x0a = wt("x0a")
                nc.vector.tensor_tensor(out=x0a[:], in0=xvb_v, in1=fx[:],
                                        op=A_OP.add)

                msk = wt("msk")
                wy0 = wt("wy0")
                bound_mask(msk, y0a, 0.0, float(H - 1))
                nc.vector.tensor_scalar(out=wy0[:], in0=ay[:], scalar1=-1.0,
                                        scalar2=1.0, op0=A_OP.mult, op1=A_OP.add)
                nc.vector.tensor_tensor(out=wy0[:], in0=wy0[:], in1=msk[:],
                                        op=A_OP.mult)
                msk2 = wt("msk")
                wy1 = wt("wy1")
                bound_mask(msk2, y0a, -1.0, float(H - 2))
                nc.vector.tensor_tensor(out=wy1[:], in0=ay[:], in1=msk2[:],
                                        op=A_OP.mult)
                msk3 = wt("msk")
                wx0 = wt("wx0")
                bound_mask(msk3, x0a, 0.0, float(W - 1))
                nc.vector.tensor_scalar(out=wx0[:], in0=ax[:], scalar1=-1.0,
                                        scalar2=1.0, op0=A_OP.mult, op1=A_OP.add)
                nc.vector.tensor_tensor(out=wx0[:], in0=wx0[:], in1=msk3[:],
                                        op=A_OP.mult)
                msk4 = wt("msk")
                wx1 = wt("wx1")
                bound_mask(msk4, x0a, -1.0, float(W - 2))
                nc.vector.tensor_tensor(out=wx1[:], in0=ax[:], in1=msk4[:],
                                        op=A_OP.mult)

                rsl = slice(roff, roff + nrows)
                nc.vector.tensor_tensor(out=sb[0][:, br, rsl], in0=wy0[:],
                                        in1=wx0[:], op=A_OP.mult)
                nc.vector.tensor_tensor(out=sb[1][:, br, rsl], in0=wy1[:],
                                        in1=wx0[:], op=A_OP.mult)
                nc.vector.tensor_tensor(out=sb[2][:, br, rsl], in0=wy0[:],
                                        in1=wx1[:], op=A_OP.mult)
                nc.vector.tensor_tensor(out=sb[3][:, br, rsl], in0=wy1[:],
                                        in1=wx1[:], op=A_OP.mult)

                idxf = wt("idxf")
                nc.vector.tensor_tensor(out=idxf[:], in0=ybrel_v, in1=fy[:],
                                        op=A_OP.add)
                nc.vector.tensor_scalar(out=idxf[:], in0=idxf[:], scalar1=float(W),
                                        scalar2=None, op0=A_OP.mult)
                nc.vector.tensor_tensor(out=idxf[:], in0=idxf[:], in1=x0a[:],
                                        op=A_OP.add)
                nc.vector.tensor_copy(idx0[:, br, rsl], idxf[:])

            _qctr = [0]

            # products engine split: list of (k, corner) -> engine
            prod_engine = []
            nact = 0
            for k in range(K):
                for corner in range(4):
                    if nact < NACT and (k * 4 + corner) % 3 != 0:
                        prod_engine.append('act')
                        nact += 1
                    else:
                        prod_engine.append('dve')

            # ---- stage 4: per-row pipeline ----
            mu3 = mpool.tile([P, NBR], F32, tag="mu3", name="mu3", padded_shape=[P, 4])
            ssq3 = mpool.tile([P, NBR], F32, tag="ssq3", name="ssq3", padded_shape=[P, 4])

            def stats_to_scale_bias(mu_t, ssq_t, n, ngrp, tag):
                """mu_t [P,ngrp] raw sums; ssq_t [P,ngrp] raw sum-of-squares.
                Returns (scale=rstd, bias=-mu*rstd) bf16-compatible f32 tiles."""
                var = mpool.tile([P, ngrp], F32, tag=tag + "v", name=tag + "v", padded_shape=[P, 4])
                # mu (mean) = mu_t/n ; var = ssq/n - mu^2 + eps
                nc.vector.tensor_scalar(out=mu_t[:], in0=mu_t[:], scalar1=1.0 / n,
                                        scalar2=None, op0=A_OP.mult)
                nc.vector.tensor_tensor(out=var[:], in0=mu_t[:], in1=mu_t[:],
                                        op=A_OP.mult)
                nc.vector.scalar_tensor_tensor(
                    out=var[:], in0=ssq_t[:], scalar=1.0 / n, in1=var[:],
                    op0=A_OP.mult, op1=A_OP.subtract)
                nc.vector.tensor_scalar(out=var[:], in0=var[:], scalar1=EPS,
                                        scalar2=None, op0=A_OP.add)
                # rstd = 1/sqrt(var)
                std = mpool.tile([P, ngrp], F32, tag=tag + "s", name=tag + "s", padded_shape=[P, 4])
                nc.scalar.activation(out=std[:], in_=var[:], func=AF.Sqrt)
                rstd = mpool.tile([P, ngrp], F32, tag=tag + "r", name=tag + "r", padded_shape=[P, 4])
                nc.vector.reciprocal(rstd[:], std[:])
                nbias = mpool.tile([P, ngrp], F32, tag=tag + "b", name=tag + "b", padded_shape=[P, 4])
                nc.vector.tensor_tensor(out=nbias[:], in0=mu_t[:], in1=rstd[:],
                                        op=A_OP.mult)
                nc.vector.tensor_scalar(out=nbias[:], in0=nbias[:], scalar1=-1.0,
                                        scalar2=None, op0=A_OP.mult)
                return rstd, nbias

            def stats_to_mu_rstd(mu_t, ssq_t, n, ngrp, tag):
                var = mpool.tile([P, ngrp], F32, tag=tag + "v", name=tag + "v", padded_shape=[P, 4])
                nc.vector.tensor_scalar(out=mu_t[:], in0=mu_t[:], scalar1=1.0 / n,
                                        scalar2=None, op0=A_OP.mult)
                nc.vector.tensor_tensor(out=var[:], in0=mu_t[:], in1=mu_t[:],
                                        op=A_OP.mult)
                nc.vector.scalar_tensor_tensor(
                    out=var[:], in0=ssq_t[:], scalar=1.0 / n, in1=var[:],
                    op0=A_OP.mult, op1=A_OP.subtract)
                nc.vector.tensor_scalar(out=var[:], in0=var[:], scalar1=EPS,
                                        scalar2=None, op0=A_OP.add)
                std = mpool.tile([P, ngrp], F32, tag=tag + "s", name=tag + "s", padded_shape=[P, 4])
                nc.scalar.activation(out=std[:], in_=var[:], func=AF.Sqrt)
                rstd = mpool.tile([P, ngrp], F32, tag=tag + "r", name=tag + "r", padded_shape=[P, 4])
                nc.vector.reciprocal(rstd[:], std[:])
                return mu_t, rstd

            for r in range(RB):
                glu = apool.tile([P, NBR * C], BF16, tag="glu")
                ysum3 = apool.tile([P, NBR, C], BF16, tag="ysum3")
                for br in range(NBR):
                    g0 = gpool.tile([P, K, 4 * C], BF16, tag="g")
                    for k in range(K):
                        gi = nc.gpsimd.indirect_dma_start(
                            out=g0[:, k, :], out_offset=None,
                            in_=d_xts[br * K + k][:],
                            in_offset=bass.IndirectOffsetOnAxis(
                                ap=idx0[:, br, r, k:k + 1], axis=0))
                        qn = _qctr[0] % 4
                        gi.queue = f"qPoolDynamic{qn if qn else ''}"
                        _qctr[0] += 1
                    # 36 in-place products: g0[:,k,corner*C:(corner+1)*C] *= s
                    for k in range(K):
                        for corner in range(4):
                            g_sl = g0[:, k, corner * C:(corner + 1) * C]
                            s_ap = sb[corner][:, br, r, k:k + 1]
                            if prod_engine[k * 4 + corner] == 'act':
                                nc.scalar.mul(g_sl, g_sl, s_ap)
                            else:
                                nc.vector.tensor_scalar(
                                    out=g_sl, in0=g_sl, scalar1=s_ap,
                                    scalar2=None, op0=A_OP.mult)
                    # corner tree on strided [P, K, C] views (innermost unit)
                    c0 = g0[:, :, 0:C]
                    c1 = g0[:, :, C:2 * C]
                    c2 = g0[:, :, 2 * C:3 * C]
                    c3 = g0[:, :, 3 * C:4 * C]
                    nc.vector.tensor_tensor(out=c0, in0=c0, in1=c1, op=A_OP.add)
                    nc.vector.tensor_tensor(out=c2, in0=c2, in1=c3, op=A_OP.add)
                    nc.vector.tensor_tensor(out=c0, in0=c0, in1=c2, op=A_OP.add)
                    # k-sum on PE: 9 identity-matmuls accumulate into PSUM
                    # (defw is pre-multiplied into the gather tables)
                    ps_df = pspool.tile([P, C], F32, tag="psdf", bufs=2)
                    for k in range(K):
                        nc.tensor.matmul(ps_df[:], lhsT=identb[:],
                                         rhs=g0[:, k, 0:C],
                                         start=(k == 0), stop=(k == K - 1))
                    # PSUM->SBUF copy, then LN sums
                    nc.scalar.activation(out=ysum3[:, br, :], in_=ps_df[:],
                                         func=AF.Copy)
                    nc.vector.tensor_reduce(
                        out=mu3[:, br:br + 1], in_=ysum3[:, br:br + 1, :],
                        op=A_OP.add, axis=mybir.AxisListType.X)
                    sqt = apool.tile([P, C], BF16, tag="sqt", bufs=2)
                    nc.vector.tensor_tensor(out=sqt[:], in0=ysum3[:, br, :],
                                            in1=ysum3[:, br, :], op=A_OP.mult)
                    nc.vector.tensor_reduce(
                        out=ssq3[:, br:br + 1],
                        in_=sqt[:].rearrange("p (g c) -> p g c", g=1),
                        op=A_OP.add, axis=mybir.AxisListType.X)

                # batched branch LN stats -> fused normalize+gelu on ACT
                rstd3, nbias3 = stats_to_scale_bias(mu3, ssq3, C, NBR, "l3")
                for br in range(NBR):
                    nc.scalar.activation(
                        out=glu[:, br * C:(br + 1) * C], in_=ysum3[:, br, :],
                        func=AF.Gelu, bias=nbias3[:, br:br + 1],
                        scale=rstd3[:, br:br + 1])

                # 1x1 conv for this row (bias added pre-transpose, per-partition)
                c1t = mpool.tile([P, C], BF16, tag="c1")
                for ct in range(2):
                    ps_c = pspool.tile([P, P], F32, tag="ps_sm", bufs=1)
                    for kt in range(2):
                        nc.tensor.matmul(
                            ps_c[:],
                            lhsT=convw[:, (kt * 2 + ct) * P:(kt * 2 + ct + 1) * P],
                            rhs=xslab[kt][:, HALO + r, HALO:HALO + P],
                            start=(kt == 0), stop=(kt == 1))
                    cb = mpool.tile([P, P], BF16, tag="cb")
                    nc.vector.tensor_scalar(out=cb[:], in0=ps_c[:],
                                            scalar1=convb[:, ct:ct + 1],
                                            scalar2=None, op0=A_OP.add)
                    ps_ct = pspool.tile([P, P], BF16, tag="ps_smb")
                    nc.tensor.transpose(ps_ct[:], cb[:], identb[:])
                    nc.any.tensor_copy(c1t[:, ct * P:(ct + 1) * P], ps_ct[:])

                # tot = glu0+glu1+glu2+c1; last add fused with running sum
                tot = mpool.tile([P, C], BF16, tag="tot")
                u01 = mpool.tile([P, C], BF16, tag="u01")
                nc.vector.tensor_tensor(out=u01[:], in0=glu[:, 0:C],
                                        in1=glu[:, C:2 * C], op=A_OP.add)
                nc.vector.tensor_tensor(out=u01[:], in0=u01[:],
                                        in1=glu[:, 2 * C:3 * C], op=A_OP.add)
                mu1 = mpool.tile([P, 1], F32, tag="mu1", name="mu1", padded_shape=[P, 2])
                nc.vector.tensor_tensor(out=tot[:], in0=u01[:], in1=c1t[:],
                                        op=A_OP.add)
                nc.vector.tensor_reduce(
                    out=mu1[:, 0:1], in_=tot[:].rearrange("p (g c) -> p g c", g=1),
                    op=A_OP.add, axis=mybir.AxisListType.X)
                ssq1 = mpool.tile([P, 1], F32, tag="ssq1", name="ssq1", padded_shape=[P, 2])
                sq1 = mpool.tile([P, C], BF16, tag="sq1", bufs=2)
                nc.vector.tensor_tensor(out=sq1[:], in0=tot[:], in1=tot[:],
                                        op=A_OP.mult)
                nc.vector.tensor_reduce(
                    out=ssq1[:, 0:1], in_=sq1[:].rearrange("p (g c) -> p g c", g=1),
                    op=A_OP.add, axis=mybir.AxisListType.X)
                mu1m, rstd1 = stats_to_mu_rstd(mu1, ssq1, C, 1, "l1")
                outr = mpool.tile([P, C], BF16, tag="outr")
                nc.vector.tensor_scalar(out=outr[:], in0=tot[:],
                                        scalar1=mu1m[:, 0:1], scalar2=rstd1[:, 0:1],
                                        op0=A_OP.subtract, op1=A_OP.mult)

                # MLP (bf16 weights)
                outT = mpool.tile([P, 2, P], BF16, tag="outT")
                for ct in range(2):
                    ps_tr = pspool.tile([P, P], BF16, tag="ps_smb")
                    nc.tensor.transpose(ps_tr[:], outr[:, ct * P:(ct + 1) * P],
                                        identb[:])
                    nc.any.tensor_copy(outT[:, ct, :], ps_tr[:])

                ps_h = pspool.tile([P, 512], F32, tag="ps2k")
                for ct in range(2):
                    nc.tensor.matmul(ps_h[:], lhsT=outT[:, ct, :],
                                     rhs=w1T[:, ct * 512:(ct + 1) * 512],
                                     start=(ct == 0), stop=False)
                nc.tensor.matmul(ps_h[:], lhsT=ones1[:], rhs=b1row[:],
                                 start=False, stop=True)
                hg = mpool.tile([P, 512], BF16, tag="hg")
                nc.scalar.activation(out=hg[:], in_=ps_h[:], func=AF.Gelu)

                hT = mpool.tile([P, 4, P], BF16, tag="hT")
                for jt in range(4):
                    ps_ht = pspool.tile([P, P], BF16, tag="ps_smb")
                    nc.tensor.transpose(ps_ht[:], hg[:, jt * P:(jt + 1) * P],
                                        identb[:])
                    nc.any.tensor_copy(hT[:, jt, :], ps_ht[:])

                ps_o = pspool.tile([P, C], F32, tag="pso", bufs=1)
                for jt in range(4):
                    nc.tensor.matmul(ps_o[:], lhsT=hT[:, jt, :],
                                     rhs=w2T[:, jt * C:(jt + 1) * C],
                                     start=(jt == 0), stop=False)
                nc.tensor.matmul(ps_o[:], lhsT=ones1[:], rhs=b2row[:],
                                 start=False, stop=True)

                # res = ps_o + outr, with running sum; then sumsq; normalize
                res = mpool.tile([P, C], BF16, tag="res")
                mu2 = mpool.tile([P, 1], F32, tag="mu2", name="mu2", padded_shape=[P, 2])
                nc.vector.tensor_tensor(out=res[:], in0=ps_o[:], in1=outr[:],
                                        op=A_OP.add)
                nc.vector.tensor_reduce(
                    out=mu2[:, 0:1], in_=res[:].rearrange("p (g c) -> p g c", g=1),
                    op=A_OP.add, axis=mybir.AxisListType.X)
                ssq2 = mpool.tile([P, 1], F32, tag="ssq2", name="ssq2", padded_shape=[P, 2])
                sq2 = mpool.tile([P, C], BF16, tag="sq2", bufs=2)
                nc.vector.tensor_tensor(out=sq2[:], in0=res[:], in1=res[:],
                                        op=A_OP.mult)
                nc.vector.tensor_reduce(
                    out=ssq2[:, 0:1], in_=sq2[:].rearrange("p (g c) -> p g c", g=1),
                    op=A_OP.add, axis=mybir.AxisListType.X)
                mu2m, rstd2 = stats_to_mu_rstd(mu2, ssq2, C, 1, "l2")
                orow = mpool.tile([P, C], F32, tag="orow")
                nc.vector.tensor_scalar(out=orow[:], in0=res[:],
                                        scalar1=mu2m[:, 0:1], scalar2=rstd2[:, 0:1],
                                        op0=A_OP.subtract, op1=A_OP.mult)
                nc.sync.dma_start(d_out[r * P:(r + 1) * P, :], orow[:])

    nc.compile()
    return nc


def stage_inputs(inputs):
    """Build per-core in_maps from full inputs (layout/dtype staging only)."""
    x = np.asarray(inputs["x"], np.float32)
    off_w = [np.asarray(inputs[f"off_w{i}"], np.float32) for i in (1, 2, 3)]
    def_w = [np.asarray(inputs[f"def_w{i}"], np.float32) for i in (1, 2, 3)]
    conv_w = np.asarray(inputs["conv_w"], np.float32)[:, :, 0, 0]
    conv_b = np.asarray(inputs["conv_b"], np.float32)
    w1 = np.asarray(inputs["mlp_w1"], np.float32)
    b1 = np.asarray(inputs["mlp_b1"], np.float32)
    w2 = np.asarray(inputs["mlp_w2"], np.float32)
    b2 = np.asarray(inputs["mlp_b2"], np.float32)

    bf = lambda a: np.ascontiguousarray(a, np.float32).astype(mybir.dt.np(BF16))
    xdt = lambda a: np.ascontiguousarray(a, np.float32).astype(
        mybir.dt.np(FP8 if USE_FP8 else BF16))

    offw = np.zeros((NBR, K, 2, P, 18), np.float32)
    for br in range(NBR):
        for tap in range(K):
            ky, kx = tap // 3, tap % 3
            for kt in range(2):
                offw[br, tap, kt] = off_w[br][:, kt * P:(kt + 1) * P, ky, kx].T
    offw_f = offw.transpose(3, 0, 1, 2, 4).reshape(P, NBR * K * 2 * 18)

    convw = np.zeros((2, 2, P, P), np.float32)
    for kt in range(2):
        for ct in range(2):
            convw[kt, ct] = conv_w[ct * P:(ct + 1) * P, kt * P:(kt + 1) * P].T
    convw_f = convw.transpose(2, 0, 1, 3).reshape(P, 4 * P)

    convb_f = np.stack([conv_b[:P], conv_b[P:]], axis=1)

    w1T_f = np.concatenate([w1.T[:P], w1.T[P:]], axis=1)        # [128, 2*512]
    w2T_f = np.concatenate([w2.T[jt * P:(jt + 1) * P] for jt in range(4)], axis=1)

    identf = np.eye(P, dtype=np.float32)
    ones1 = np.ones((1, P), np.float32)

    shared = dict(
        offw=bf(offw_f), convw=bf(convw_f), convb=convb_f,
        w1T=bf(w1T_f), b1row=bf(b1[None, :]), w2T=bf(w2T_f), b2row=bf(b2[None, :]),
        identf=identf, identb=bf(identf), ones1=bf(ones1),
    )

    in_maps = []
    xr = x.reshape(B, 2, P, H, W)
    for core in range(NCORES):
        b = core // 4
        y0 = (core % 4) * RB
        slab = np.zeros((2, P, SROWS, WP), np.float32)
        rlo, rhi = y0 - HALO, y0 + RB + HALO
        srlo, srhi = max(rlo, 0), min(rhi, H)
        slab[:, :, srlo - rlo:srhi - rlo, HALO:HALO + W] = xr[b][:, :, srlo:srhi]
        xT1 = np.zeros((TR_ALLOC + W, C), np.float32)
        tlo = y0 - TPAD
        alo, ahi = max(tlo, 0), min(y0 + RB + TPAD + 1, H)
        xT1[(alo - tlo) * W:(ahi - tlo) * W] = \
            x[b, :, alo:ahi, :].reshape(C, -1).T
        xT = np.concatenate([xT1[:TR_ALLOC], xT1[W:TR_ALLOC + W]], axis=1)
        xts = {}
        for br in range(NBR):
            dwk = def_w[br][:, 0].reshape(C, K)
            for k in range(K):
                pat = np.concatenate([dwk[:, k], dwk[:, k]])
                xts[f"xt{br * K + k}"] = xdt(xT * pat[None, :])
        ybrel = np.zeros((P, NBR, RB, K), np.float32)
        ybabs = np.zeros((P, NBR, RB, K), np.float32)
        xvb = np.zeros((P, NBR, RB, K), np.float32)
        for br in range(NBR):
            dil = DILS[br]
            for k in range(K):
                ky, kx = k // 3, k % 3
                rows = y0 + np.arange(RB) + (ky - 1) * dil
                ybabs[:, br, :, k] = rows[None, :]
                ybrel[:, br, :, k] = rows[None, :] - y0 + TPAD
                xvb[:, br, :, k] = (np.arange(P) + (kx - 1) * dil)[:, None]
        m = dict(shared)
        m.update(xts)
        m.update(xslab=bf(slab.reshape(2, P, SROWS * WP)),
                 ybrel=bf(ybrel.reshape(P, -1)), ybabs=bf(ybabs.reshape(P, -1)),
                 xvb=bf(xvb.reshape(P, -1)))
        in_maps.append(m)
    return in_maps


def assemble_output(results):
    out = np.zeros((B, C, H, W), np.float32)
    for core in range(NCORES):
        b = core // 4
        y0 = (core % 4) * RB
        o = np.asarray(results[core]["out"], np.float32)
        out[b, :, y0:y0 + RB, :] = o.reshape(RB, W, C).transpose(2, 0, 1)
    return out


def kernel(**inputs):
    global _COMPILED
    from concourse.bass_utils import run_bass_kernel_spmd
    if _COMPILED is None:
        _COMPILED = build_program()
    nc = _COMPILED
    in_maps = stage_inputs(inputs)
    res = run_bass_kernel_spmd(nc, in_maps, core_ids=list(range(NCORES)))
    return assemble_output(res.results)
